# revision 24
# baseline (speedup 1.0000x reference)
"""Fused DeepFeatureLoss kernel for 8 Trainium2 NeuronCores.

Reference computation (per batch b, N=4096 points, D=32 features):
    pd[i,j] = -||p_i - p_j||^2 / sigma^2          (points, sigma=0.005)
    fd[i,j] = -||f1_i - f2_j||^2
    ce[i]   = -sum_j softmax(pd)[i,j] * log_softmax(fd)[i,j]
    ce_loss[b]  = sum_i ce[i] * w[i]
    reg_loss[b] = mean_{i, c>=3} (f1[i,c]^2 + f2[i,c]^2)

Identities used:
    ce[i] = log(Zf_i) - S_i / Zp_i
    Zp_i = sum_j exp(pd[i,j]);  Zf_i = sum_j exp(fd[i,j]);  S_i = sum_j exp(pd[i,j]) * fd[i,j]
(log_softmax is shift-invariant per row; both pd and fd are negative squared
distances, so exp never overflows and no max-subtraction pass is needed.)

Gaussian banding: with sigma=0.005, exp(pd) underflows to exactly 0.0f for
point distances > 0.047. The host sorts each batch's rows by Morton code of
the points (an exact permutation — all row/column sums here are permutation
invariant), after which every non-underflowing pair sits within +/-192 of the
diagonal (verified: zero leaked pairs on the reference data, <1e-5 loss error
across random re-seeds). So the point-softmax (exp, Zp, S) runs on a 512-wide
band per 128-row block instead of all 4096 columns. The feature log-softmax
still needs full rows (for Zf), so fd is computed densely; the 512 band
columns of fd needed for S are recomputed by a tiny extra matmul so that
every PSUM tile has exactly one reader (Tile serializes same-bank PSUM
readers otherwise).

Both distance matrices are produced directly in PSUM by augmented matmuls:
    pd = A_p @ B_p^T, A_p[i] = (2*p_i/s^2, 1, |p_i|^2/s^2), B_p[j] = (p_j, -|p_j|^2/s^2, -1)  (K=5,  fp32)
    fd = A_f @ B_f^T, A_f[i] = (2*f1_i, 1, |f1_i|^2),       B_f[j] = (f2_j, -|f2_j|^2, -1)    (K=34, fp32r)

Engine balance per core: ScalarE's exps are the hard floor (~31us of data).
Every exp writes its (dead) result back into the PSUM tile it reads and
accumulates its row sum on ScalarE itself, so VectorE only carries the band
product-sum, the reg sums and the O(RB) finalize. Matmuls alternate between
two stationary row-group placements (A rows 0..33, B rows 64..97) so the
LDWEIGHTS of each matmul overlaps the previous matmul. The feature sweep is
chunk-outer so each column slab of b-operand DMA is consumed right after it
lands; the band work rides inside the last chunk's sweep.

Sharding: core k handles batch k//4, rows [1024*(k%4), +1024) of the sorted
order. Each core emits 16 partials (8 ce cols + 8 reg cols, partition-reduced
on PE with a ones-vector); the host adds them up.
"""

import ml_dtypes
import numpy as np
from contextlib import ExitStack

import concourse.bacc as bacc
import concourse.bass as bass
import concourse.tile as tile
from concourse import mybir
from concourse.bass_utils import run_bass_kernel_spmd

SIGMA = 0.005
B, N, D = 2, 4096, 32
NCORES = 8
CPB = NCORES // B            # cores per batch = 4
ROWS = N // CPB              # rows per core = 1024
RB = ROWS // 128             # 128-row blocks per core = 8
NFC = 4                      # fd chunks per row block
FCH = N // NFC               # fd chunk width = 1024 (2 PSUM banks)
W = 512                      # point-band width
PAD = (W - 128) // 2         # 192: band = [g0-192, g0+320) clamped
KP = 5                       # augmented K for points
KF = D + 2                   # augmented K for features = 34
F32 = mybir.dt.float32
F32R = mybir.dt.float32r
BF16 = mybir.dt.bfloat16

_CACHE = {}


def _build():
    nc = bacc.Bacc(trn_type="TRN2")
    aptT = nc.declare_dram_parameter("aptT", [KP, ROWS], F32, isOutput=False)
    bptb = nc.declare_dram_parameter("bptb", [KP, RB * W], F32, isOutput=False)
    afeT = nc.declare_dram_parameter("afeT", [KF, ROWS], BF16, isOutput=False)
    bfeT = nc.declare_dram_parameter("bfeT", [KF, N], BF16, isOutput=False)
    afeR = nc.declare_dram_parameter("afeR", [KF, ROWS], F32R, isOutput=False)
    bfeb = nc.declare_dram_parameter("bfeb", [KF, RB * W], F32R, isOutput=False)
    wcol = nc.declare_dram_parameter("wcol", [128, RB], F32, isOutput=False)
    f1r = nc.declare_dram_parameter("f1r", [128, RB * D], F32, isOutput=False)
    f2r = nc.declare_dram_parameter("f2r", [128, RB * D], F32, isOutput=False)
    outp = nc.declare_dram_parameter("partials", [1, 16], F32, isOutput=True)

    AF = mybir.ActivationFunctionType
    OP = mybir.AluOpType

    with ExitStack() as ctx:
        tc = ctx.enter_context(tile.TileContext(nc))
        singles = ctx.enter_context(tc.tile_pool(name="singles", bufs=1))
        fd_pool = ctx.enter_context(tc.tile_pool(name="fdp", bufs=4, space="PSUM"))
        pdb_pool = fd_pool
        fdb_pool = fd_pool
        ep_pool = ctx.enter_context(tc.tile_pool(name="epp", bufs=2))
        ef_pool = ctx.enter_context(tc.tile_pool(name="efp", bufs=2))
        stt_pool = ctx.enter_context(tc.tile_pool(name="sttp", bufs=2))
        r29_pool = ctx.enter_context(tc.tile_pool(name="r29p", bufs=2))

        # ---- input loads, ordered by first use: sync HWDGE for the
        # compute-critical operands, gpsimd for the late reg-only ones ----
        afe_sb = singles.tile([128, ROWS], BF16)
        bfe_sb = singles.tile([128, N], BF16)
        # the four descriptors the first matmuls depend on go out on the
        # otherwise-idle ScalarE queue, in parallel with the sync queue's
        nc.scalar.dma_start(out=afe_sb[0:KF, :], in_=afeT[:, :])
        nc.scalar.dma_start(out=afe_sb[64 : 64 + KF, :], in_=afeT[:, :])
        nc.scalar.dma_start(out=bfe_sb[0:KF, 0:FCH], in_=bfeT[:, 0:FCH])
        nc.scalar.dma_start(out=bfe_sb[64 : 64 + KF, 0:FCH], in_=bfeT[:, 0:FCH])

        def load_bfe(h):
            nc.sync.dma_start(
                out=bfe_sb[0:KF, h * FCH : (h + 1) * FCH],
                in_=bfeT[:, h * FCH : (h + 1) * FCH],
            )
            nc.sync.dma_start(
                out=bfe_sb[64 : 64 + KF, h * FCH : (h + 1) * FCH],
                in_=bfeT[:, h * FCH : (h + 1) * FCH],
            )

        load_bfe(1)
        load_bfe(2)
        afr_sb = singles.tile([KF, ROWS], F32R)
        nc.sync.dma_start(out=afr_sb[:, :], in_=afeR[:, :])
        apt_sb = singles.tile([128, ROWS], F32)
        nc.sync.dma_start(out=apt_sb[96 : 96 + KP, :], in_=aptT[:, :])
        nc.sync.dma_start(out=apt_sb[64 : 64 + KP, :], in_=aptT[:, :])
        bpt_sb = singles.tile([128, RB * W], F32)
        nc.sync.dma_start(out=bpt_sb[96 : 96 + KP, :], in_=bptb[:, :])
        nc.sync.dma_start(out=bpt_sb[64 : 64 + KP, :], in_=bptb[:, :])
        bfb_sb = singles.tile([KF, RB * W], F32R)
        nc.sync.dma_start(out=bfb_sb[:, :], in_=bfeb[:, :])
        load_bfe(3)
        # reg-only operands: gpsimd SWDGE keeps them off the critical path
        w_sb = singles.tile([128, RB], F32)
        nc.gpsimd.dma_start(out=w_sb[:, :], in_=wcol[:, :])
        f1_sb = singles.tile([128, RB * D], F32)
        nc.gpsimd.dma_start(out=f1_sb[:, :], in_=f1r[:, :])
        f2_sb = singles.tile([128, RB * D], F32)
        nc.gpsimd.dma_start(out=f2_sb[:, :], in_=f2r[:, :])

        # per-block statistics
        zf4 = singles.tile([128, RB * NFC], F32)   # col rb*4+c
        zp1 = singles.tile([128, RB], F32)
        sp1 = singles.tile([128, RB], F32)
        rg1 = singles.tile([128, RB], F32)
        rg2 = singles.tile([128, RB], F32)
        ones_sb = singles.tile([128, 1], F32)
        nc.vector.memset(ones_sb, 1.0)

        def fd_chunk(rb, c):
            """Full feature-distance rows for block rb, columns [c*1024, +1024).
            exp runs in place on the PSUM tile, row sum stays on ScalarE."""
            r0 = rb * 128
            fdc = fd_pool.tile([128, FCH], F32, tag="fdc", name=f"fd_{rb}_{c}")
            j0 = c * FCH
            nc.tensor.matmul(
                fdc[:, 0:512],
                lhsT=afe_sb[0:KF, r0 : r0 + 128],
                rhs=bfe_sb[0:KF, j0 : j0 + 512],
                start=True,
                stop=True,
            )
            nc.tensor.matmul(
                fdc[:, 512:1024],
                lhsT=afe_sb[64 : 64 + KF, r0 : r0 + 128],
                rhs=bfe_sb[64 : 64 + KF, j0 + 512 : j0 + 1024],
                start=True,
                stop=True,
                tile_position=(64, 0),
            )
            zcol = zf4[:, rb * NFC + c : rb * NFC + c + 1]
            if c < 2:
                nc.scalar.activation(
                    out=fdc[:, :], in_=fdc[:, :], func=AF.Exp, accum_out=zcol
                )
            else:
                ef = ef_pool.tile([128, FCH], BF16, tag="ef")
                nc.scalar.activation(out=ef, in_=fdc[:, :], func=AF.Exp)
                nc.vector.tensor_reduce(
                    out=zcol, in_=ef, axis=mybir.AxisListType.X, op=OP.add
                )

        for c in range(NFC - 1):
            for rb in range(RB):
                fd_chunk(rb, c)

        # last chunk sweep with the band work riding along
        for rb in range(RB):
            fd_chunk(rb, NFC - 1)
            r0 = rb * 128
            # feature band (the 512 columns the point softmax needs)
            fdb_t = fdb_pool.tile([128, FCH], F32, tag="fdc", name=f"fdb_{rb}")
            fdb = fdb_t[:, 0:W]
            nc.tensor.matmul(
                fdb[:, :],
                lhsT=afr_sb[0:KF, r0 : r0 + 128],
                rhs=bfb_sb[0:KF, rb * W : (rb + 1) * W],
                start=True,
                stop=True,
            )
            # point band
            pb_base = 96 if rb % 2 == 0 else 64
            pdb_t = pdb_pool.tile([128, FCH], F32, tag="fdc", name=f"pdb_{rb}")
            pdb = pdb_t[:, 0:W]
            nc.tensor.matmul(
                pdb[:, :],
                lhsT=apt_sb[pb_base : pb_base + KP, r0 : r0 + 128],
                rhs=bpt_sb[pb_base : pb_base + KP, rb * W : (rb + 1) * W],
                start=True,
                stop=True,
                tile_position=(pb_base, 0),
            )
            ep = ep_pool.tile([128, W], F32, tag="ep")
            nc.scalar.activation(
                out=ep, in_=pdb[:, :], func=AF.Exp, accum_out=zp1[:, rb : rb + 1]
            )
            stt = stt_pool.tile([128, W], F32, tag="stt")
            nc.vector.scalar_tensor_tensor(
                out=stt,
                in0=ep,
                scalar=1.0,
                in1=fdb[:, :],
                op0=OP.mult,
                op1=OP.mult,
                accum_out=sp1[:, rb : rb + 1],
            )

        # reg partials on DVE: sum_{c>=3} f^2 per row
        for rb in range(RB):
            sA = r29_pool.tile([128, D - 3], F32, tag="r29")
            nc.vector.scalar_tensor_tensor(
                out=sA,
                in0=f1_sb[:, rb * D + 3 : (rb + 1) * D],
                scalar=1.0,
                in1=f1_sb[:, rb * D + 3 : (rb + 1) * D],
                op0=OP.mult,
                op1=OP.mult,
                accum_out=rg1[:, rb : rb + 1],
            )
            sB = r29_pool.tile([128, D - 3], F32, tag="r29")
            nc.vector.scalar_tensor_tensor(
                out=sB,
                in0=f2_sb[:, rb * D + 3 : (rb + 1) * D],
                scalar=1.0,
                in1=f2_sb[:, rb * D + 3 : (rb + 1) * D],
                op0=OP.mult,
                op1=OP.mult,
                accum_out=rg2[:, rb : rb + 1],
            )

        # ---- finalize: ce = w * (ln(Zf) - S/Zp), reduce over all rows ----
        zf_all = singles.tile([128, RB], F32)
        nc.vector.tensor_reduce(
            out=zf_all,
            in_=zf4.rearrange("p (r c) -> p r c", r=RB),
            axis=mybir.AxisListType.X,
            op=OP.add,
        )
        lse = singles.tile([128, RB], F32)
        nc.scalar.activation(out=lse, in_=zf_all, func=AF.Ln)
        rzp = singles.tile([128, RB], F32)
        nc.vector.reciprocal(out=rzp, in_=zp1)
        t1 = singles.tile([128, RB], F32)
        nc.vector.tensor_mul(t1, sp1, rzp)
        ce_all = singles.tile([128, RB], F32)
        nc.vector.tensor_sub(ce_all, lse, t1)
        wce = singles.tile([128, RB], F32)
        nc.vector.tensor_mul(wce, ce_all, w_sb)
        rg = singles.tile([128, RB], F32)
        nc.vector.tensor_add(rg, rg1, rg2)

        # partition-reduce [128, 8+8] -> [1, 16] with a ones matmul
        red = pdb_pool.tile([128, FCH], F32, tag="fdc", name="red")
        nc.tensor.matmul(
            red[0:1, 0:RB], lhsT=ones_sb[:, 0:1], rhs=wce, start=True, stop=True
        )
        nc.tensor.matmul(
            red[0:1, RB : 2 * RB], lhsT=ones_sb[:, 0:1], rhs=rg, start=True, stop=True
        )
        out_sb = singles.tile([1, 16], F32)
        nc.vector.tensor_copy(out=out_sb[0:1, :], in_=red[0:1, 0 : 2 * RB])
        nc.sync.dma_start(out=outp[:, :], in_=out_sb[:, :])
    return nc


def _morton(p, bits=10):
    q = np.minimum((p * (1 << bits)).astype(np.uint64), (1 << bits) - 1)
    code = np.zeros(len(p), np.uint64)
    for b in range(bits):
        for dim in range(3):
            code |= ((q[:, dim] >> np.uint64(b)) & np.uint64(1)) << np.uint64(3 * b + dim)
    return code


def _fp22(x):
    return (x.view(np.uint32) & np.uint32(0xFFFFFC00)).view(np.float32)


def _prep_batch(b, points, pointfea1, pointfea2, weights):
    perm = np.argsort(_morton(points[b]))
    inv = np.float32(1.0 / (SIGMA * SIGMA))
    p = points[b][perm]
    f1 = pointfea1[b][perm]
    f2 = pointfea2[b][perm]
    w = weights[b, :, 0][perm]

    p2 = (p * p).sum(1)
    f1sq = (f1 * f1).sum(1)
    f2sq = (f2 * f2).sum(1)
    onesN = np.ones((N, 1), np.float32)

    a_pts = np.concatenate([2.0 * p * inv, onesN, (p2 * inv)[:, None]], 1).astype(np.float32)
    b_pts = np.concatenate([p, -(p2 * inv)[:, None], -onesN], 1).astype(np.float32)
    a_fea = _fp22(np.concatenate([2.0 * f1, onesN, f1sq[:, None]], 1).astype(np.float32))
    b_fea = _fp22(np.concatenate([f2, -f2sq[:, None], -onesN], 1).astype(np.float32))
    a_fea_bf = a_fea.astype(ml_dtypes.bfloat16)
    b_fea_bf = b_fea.astype(ml_dtypes.bfloat16)
    return p, f1, f2, w, a_pts, b_pts, a_fea, b_fea, a_fea_bf, b_fea_bf


def make_in_maps(points, pointfea1, pointfea2, weights):
    points = np.asarray(points, np.float32)
    pointfea1 = np.asarray(pointfea1, np.float32)
    pointfea2 = np.asarray(pointfea2, np.float32)
    weights = np.asarray(weights, np.float32)

    batch_data = [
        _prep_batch(b, points, pointfea1, pointfea2, weights) for b in range(B)
    ]
    in_maps = []
    for k in range(NCORES):
        b = k // CPB
        r0 = (k % CPB) * ROWS
        p, f1, f2, w, a_pts, b_pts, a_fea, b_fea, a_fea_bf, b_fea_bf = batch_data[b]
        # per-row-block band starts (global j), gathered host-side
        bpt_band = np.empty((KP, RB * W), np.float32)
        bfe_band = np.empty((KF, RB * W), np.float32)
        for rb in range(RB):
            g0 = r0 + rb * 128
            s = min(max(g0 - PAD, 0), N - W)
            bpt_band[:, rb * W : (rb + 1) * W] = b_pts[s : s + W].T
            bfe_band[:, rb * W : (rb + 1) * W] = b_fea[s : s + W].T
        in_maps.append(
            {
                "aptT": np.ascontiguousarray(a_pts[r0 : r0 + ROWS].T),
                "bptb": bpt_band,
                "afeT": np.ascontiguousarray(a_fea_bf[r0 : r0 + ROWS].T),
                "bfeT": np.ascontiguousarray(b_fea_bf.T),
                "afeR": np.ascontiguousarray(a_fea[r0 : r0 + ROWS].T),
                "bfeb": bfe_band,
                "wcol": np.ascontiguousarray(w[r0 : r0 + ROWS].reshape(RB, 128).T),
                "f1r": np.ascontiguousarray(
                    f1[r0 : r0 + ROWS].reshape(RB, 128, D).transpose(1, 0, 2).reshape(128, RB * D)
                ),
                "f2r": np.ascontiguousarray(
                    f2[r0 : r0 + ROWS].reshape(RB, 128, D).transpose(1, 0, 2).reshape(128, RB * D)
                ),
            }
        )
    return in_maps


def get_nc():
    if "nc" not in _CACHE:
        nc = _build()
        nc.finalize()
        _CACHE["nc"] = nc
    return _CACHE["nc"]


def combine_partials(parts):
    """parts: [NCORES, 16] array of per-core (8 ce cols, 8 reg cols)."""
    parts = np.asarray(parts, np.float64)
    ce = parts[:, 0:RB].sum(1).reshape(B, CPB).sum(1)
    reg = parts[:, RB : 2 * RB].sum(1).reshape(B, CPB).sum(1) / (29.0 * N)
    return ce.astype(np.float32), reg.astype(np.float32)


def kernel(points, pointfea1, pointfea2, weights):
    nc = get_nc()
    in_maps = make_in_maps(points, pointfea1, pointfea2, weights)
    res = run_bass_kernel_spmd(nc, in_maps, core_ids=list(range(NCORES)))
    parts = np.stack([res.results[k]["partials"][0] for k in range(NCORES)])
    return combine_partials(parts)


# revision 25
# speedup vs baseline: 1.1734x; 1.1734x over previous
"""Fused DeepFeatureLoss kernel for 8 Trainium2 NeuronCores.

Reference computation (per batch b, N=4096 points, D=32 features):
    pd[i,j] = -||p_i - p_j||^2 / sigma^2          (points, sigma=0.005)
    fd[i,j] = -||f1_i - f2_j||^2
    ce[i]   = -sum_j softmax(pd)[i,j] * log_softmax(fd)[i,j]
    ce_loss[b]  = sum_i ce[i] * w[i]
    reg_loss[b] = mean_{i, c>=3} (f1[i,c]^2 + f2[i,c]^2)

Identities used:
    ce[i] = log(Zf_i) - S_i / Zp_i
    Zp_i = sum_j exp(pd[i,j]);  Zf_i = sum_j exp(fd[i,j]);  S_i = sum_j exp(pd[i,j]) * fd[i,j]
(log_softmax is shift-invariant per row; both pd and fd are negative squared
distances, so exp never overflows and no max-subtraction pass is needed.)

Gaussian banding: with sigma=0.005, exp(pd) underflows to exactly 0.0f for
point distances > 0.047. The host sorts each batch's rows by Morton code of
the points (an exact permutation — all row/column sums here are permutation
invariant), after which every non-underflowing pair sits within +/-192 of the
diagonal (verified: zero leaked pairs on the reference data, <1e-5 loss error
across random re-seeds). So the point-softmax (exp, Zp, S) runs on a 512-wide
band per 128-row block instead of all 4096 columns. The feature log-softmax
still needs full rows (for Zf), so fd is computed densely; the 512 band
columns of fd needed for S are recomputed by a tiny extra matmul so that
every PSUM tile has exactly one reader (Tile serializes same-bank PSUM
readers otherwise).

Both distance matrices are produced directly in PSUM by augmented matmuls:
    pd = A_p @ B_p^T, A_p[i] = (2*p_i/s^2, 1, |p_i|^2/s^2), B_p[j] = (p_j, -|p_j|^2/s^2, -1)  (K=5,  fp32)
    fd = A_f @ B_f^T, A_f[i] = (2*f1_i, 1, |f1_i|^2),       B_f[j] = (f2_j, -|f2_j|^2, -1)    (K=34, fp32r)

Engine balance per core: ScalarE's exps are the hard floor (~31us of data).
Every exp writes its (dead) result back into the PSUM tile it reads and
accumulates its row sum on ScalarE itself, so VectorE only carries the band
product-sum, the reg sums and the O(RB) finalize. Matmuls alternate between
two stationary row-group placements (A rows 0..33, B rows 64..97) so the
LDWEIGHTS of each matmul overlaps the previous matmul. The feature sweep is
chunk-outer so each column slab of b-operand DMA is consumed right after it
lands; the band work rides inside the last chunk's sweep.

Sharding: core k handles batch k//4, rows [1024*(k%4), +1024) of the sorted
order. Each core emits 16 partials (8 ce cols + 8 reg cols, partition-reduced
on PE with a ones-vector); the host adds them up.
"""

import ml_dtypes
import numpy as np
from contextlib import ExitStack

import concourse.bacc as bacc
import concourse.bass as bass
import concourse.tile as tile
from concourse import mybir
from concourse.bass_utils import run_bass_kernel_spmd

SIGMA = 0.005
B, N, D = 2, 4096, 32
NCORES = 8
CPB = NCORES // B            # cores per batch = 4
ROWS = N // CPB              # rows per core = 1024
RB = ROWS // 128             # 128-row blocks per core = 8
NFC = 4                      # fd chunks per row block
FCH = N // NFC               # fd chunk width = 1024 (2 PSUM banks)
W = 512                      # point-band width
PAD = (W - 128) // 2         # 192: band = [g0-192, g0+320) clamped
KP = 5                       # augmented K for points
KF = D + 2                   # augmented K for features = 34
F32 = mybir.dt.float32
F32R = mybir.dt.float32r
BF16 = mybir.dt.bfloat16

_CACHE = {}


def _build():
    nc = bacc.Bacc(trn_type="TRN2")
    aptT = nc.declare_dram_parameter("aptT", [KP, ROWS], F32, isOutput=False)
    bptb = nc.declare_dram_parameter("bptb", [KP, RB * W], F32, isOutput=False)
    afeT = nc.declare_dram_parameter("afeT", [KF, ROWS], BF16, isOutput=False)
    bfeT = nc.declare_dram_parameter("bfeT", [KF, N], BF16, isOutput=False)
    afeR = nc.declare_dram_parameter("afeR", [KF, ROWS], F32R, isOutput=False)
    bfeb = nc.declare_dram_parameter("bfeb", [KF, RB * W], F32R, isOutput=False)
    wcol = nc.declare_dram_parameter("wcol", [128, RB], F32, isOutput=False)
    f1r = nc.declare_dram_parameter("f1r", [128, RB * D], F32, isOutput=False)
    f2r = nc.declare_dram_parameter("f2r", [128, RB * D], F32, isOutput=False)
    outp = nc.declare_dram_parameter("partials", [1, 16], F32, isOutput=True)

    AF = mybir.ActivationFunctionType
    OP = mybir.AluOpType

    with ExitStack() as ctx:
        tc = ctx.enter_context(tile.TileContext(nc))
        singles = ctx.enter_context(tc.tile_pool(name="singles", bufs=1))
        fd_pool = ctx.enter_context(tc.tile_pool(name="fdp", bufs=2, space="PSUM"))
        pdb_pool = ctx.enter_context(tc.tile_pool(name="pdbp", bufs=2, space="PSUM"))
        fdb_pool = ctx.enter_context(tc.tile_pool(name="fdbp", bufs=2, space="PSUM"))
        ep_pool = ctx.enter_context(tc.tile_pool(name="epp", bufs=2))
        ef_pool = ctx.enter_context(tc.tile_pool(name="efp", bufs=2))
        stt_pool = ctx.enter_context(tc.tile_pool(name="sttp", bufs=2))
        r29_pool = ctx.enter_context(tc.tile_pool(name="r29p", bufs=2))

        # ---- input loads, ordered by first use: sync HWDGE for the
        # compute-critical operands, gpsimd for the late reg-only ones ----
        afe_sb = singles.tile([128, ROWS], BF16)
        nc.sync.dma_start(out=afe_sb[0:KF, :], in_=afeT[:, :])
        nc.sync.dma_start(out=afe_sb[64 : 64 + KF, :], in_=afeT[:, :])
        bfe_sb = singles.tile([128, N], BF16)

        def load_bfe(h):
            nc.sync.dma_start(
                out=bfe_sb[0:KF, h * FCH : (h + 1) * FCH],
                in_=bfeT[:, h * FCH : (h + 1) * FCH],
            )
            nc.sync.dma_start(
                out=bfe_sb[64 : 64 + KF, h * FCH : (h + 1) * FCH],
                in_=bfeT[:, h * FCH : (h + 1) * FCH],
            )

        load_bfe(0)
        load_bfe(1)
        load_bfe(2)
        afr_sb = singles.tile([KF, ROWS], F32R)
        nc.sync.dma_start(out=afr_sb[:, :], in_=afeR[:, :])
        apt_sb = singles.tile([128, ROWS], F32)
        nc.sync.dma_start(out=apt_sb[96 : 96 + KP, :], in_=aptT[:, :])
        nc.sync.dma_start(out=apt_sb[64 : 64 + KP, :], in_=aptT[:, :])
        bpt_sb = singles.tile([128, RB * W], F32)
        nc.sync.dma_start(out=bpt_sb[96 : 96 + KP, :], in_=bptb[:, :])
        nc.sync.dma_start(out=bpt_sb[64 : 64 + KP, :], in_=bptb[:, :])
        bfb_sb = singles.tile([KF, RB * W], F32R)
        nc.sync.dma_start(out=bfb_sb[:, :], in_=bfeb[:, :])
        load_bfe(3)
        # reg-only operands: gpsimd SWDGE keeps them off the critical path
        w_sb = singles.tile([128, RB], F32)
        nc.gpsimd.dma_start(out=w_sb[:, :], in_=wcol[:, :])
        f1_sb = singles.tile([128, RB * D], F32)
        nc.gpsimd.dma_start(out=f1_sb[:, :], in_=f1r[:, :])
        f2_sb = singles.tile([128, RB * D], F32)
        nc.gpsimd.dma_start(out=f2_sb[:, :], in_=f2r[:, :])

        # per-block statistics
        zf4 = singles.tile([128, RB * NFC], F32)   # col rb*4+c
        zp1 = singles.tile([128, RB], F32)
        sp1 = singles.tile([128, RB], F32)
        rg1 = singles.tile([128, RB], F32)
        rg2 = singles.tile([128, RB], F32)
        ones_sb = singles.tile([128, 1], F32)
        nc.vector.memset(ones_sb, 1.0)

        def fd_chunk(rb, c):
            """Full feature-distance rows for block rb, columns [c*1024, +1024).
            exp runs in place on the PSUM tile, row sum stays on ScalarE."""
            r0 = rb * 128
            fdc = fd_pool.tile([128, FCH], F32, tag="fdc", name=f"fd_{rb}_{c}")
            j0 = c * FCH
            nc.tensor.matmul(
                fdc[:, 0:512],
                lhsT=afe_sb[0:KF, r0 : r0 + 128],
                rhs=bfe_sb[0:KF, j0 : j0 + 512],
                start=True,
                stop=True,
            )
            nc.tensor.matmul(
                fdc[:, 512:1024],
                lhsT=afe_sb[64 : 64 + KF, r0 : r0 + 128],
                rhs=bfe_sb[64 : 64 + KF, j0 + 512 : j0 + 1024],
                start=True,
                stop=True,
                tile_position=(64, 0),
            )
            zcol = zf4[:, rb * NFC + c : rb * NFC + c + 1]
            if c < 2:
                nc.scalar.activation(
                    out=fdc[:, :], in_=fdc[:, :], func=AF.Exp, accum_out=zcol
                )
            else:
                ef = ef_pool.tile([128, FCH], BF16, tag="ef")
                nc.scalar.activation(out=ef, in_=fdc[:, :], func=AF.Exp)
                nc.vector.tensor_reduce(
                    out=zcol, in_=ef, axis=mybir.AxisListType.X, op=OP.add
                )

        for c in range(NFC - 1):
            for rb in range(RB):
                fd_chunk(rb, c)

        # last chunk sweep with the band work riding along
        for rb in range(RB):
            fd_chunk(rb, NFC - 1)
            r0 = rb * 128
            # feature band (the 512 columns the point softmax needs)
            fdb = fdb_pool.tile([128, W], F32, tag="fdb", name=f"fdb_{rb}")
            nc.tensor.matmul(
                fdb[:, :],
                lhsT=afr_sb[0:KF, r0 : r0 + 128],
                rhs=bfb_sb[0:KF, rb * W : (rb + 1) * W],
                start=True,
                stop=True,
            )
            # point band
            pb_base = 96 if rb % 2 == 0 else 64
            pdb = pdb_pool.tile([128, W], F32, tag="pdb", name=f"pdb_{rb}")
            nc.tensor.matmul(
                pdb[:, :],
                lhsT=apt_sb[pb_base : pb_base + KP, r0 : r0 + 128],
                rhs=bpt_sb[pb_base : pb_base + KP, rb * W : (rb + 1) * W],
                start=True,
                stop=True,
                tile_position=(pb_base, 0),
            )
            ep = ep_pool.tile([128, W], F32, tag="ep")
            nc.scalar.activation(
                out=ep, in_=pdb[:, :], func=AF.Exp, accum_out=zp1[:, rb : rb + 1]
            )
            stt = stt_pool.tile([128, W], F32, tag="stt")
            nc.vector.scalar_tensor_tensor(
                out=stt,
                in0=ep,
                scalar=1.0,
                in1=fdb[:, :],
                op0=OP.mult,
                op1=OP.mult,
                accum_out=sp1[:, rb : rb + 1],
            )

        # reg partials on DVE: sum_{c>=3} f^2 per row
        for rb in range(RB):
            sA = r29_pool.tile([128, D - 3], F32, tag="r29")
            nc.vector.scalar_tensor_tensor(
                out=sA,
                in0=f1_sb[:, rb * D + 3 : (rb + 1) * D],
                scalar=1.0,
                in1=f1_sb[:, rb * D + 3 : (rb + 1) * D],
                op0=OP.mult,
                op1=OP.mult,
                accum_out=rg1[:, rb : rb + 1],
            )
            sB = r29_pool.tile([128, D - 3], F32, tag="r29")
            nc.vector.scalar_tensor_tensor(
                out=sB,
                in0=f2_sb[:, rb * D + 3 : (rb + 1) * D],
                scalar=1.0,
                in1=f2_sb[:, rb * D + 3 : (rb + 1) * D],
                op0=OP.mult,
                op1=OP.mult,
                accum_out=rg2[:, rb : rb + 1],
            )

        # ---- finalize: ce = w * (ln(Zf) - S/Zp), reduce over all rows ----
        zf_all = singles.tile([128, RB], F32)
        nc.vector.tensor_reduce(
            out=zf_all,
            in_=zf4.rearrange("p (r c) -> p r c", r=RB),
            axis=mybir.AxisListType.X,
            op=OP.add,
        )
        lse = singles.tile([128, RB], F32)
        nc.scalar.activation(out=lse, in_=zf_all, func=AF.Ln)
        rzp = singles.tile([128, RB], F32)
        nc.vector.reciprocal(out=rzp, in_=zp1)
        t1 = singles.tile([128, RB], F32)
        nc.vector.tensor_mul(t1, sp1, rzp)
        ce_all = singles.tile([128, RB], F32)
        nc.vector.tensor_sub(ce_all, lse, t1)
        wce = singles.tile([128, RB], F32)
        nc.vector.tensor_mul(wce, ce_all, w_sb)
        rg = singles.tile([128, RB], F32)
        nc.vector.tensor_add(rg, rg1, rg2)

        # partition-reduce [128, 8+8] -> [1, 16] with a ones matmul
        red = pdb_pool.tile([128, W], F32, tag="pdb", name="red")
        nc.tensor.matmul(
            red[0:1, 0:RB], lhsT=ones_sb[:, 0:1], rhs=wce, start=True, stop=True
        )
        nc.tensor.matmul(
            red[0:1, RB : 2 * RB], lhsT=ones_sb[:, 0:1], rhs=rg, start=True, stop=True
        )
        out_sb = singles.tile([1, 16], F32)
        nc.vector.tensor_copy(out=out_sb[0:1, :], in_=red[0:1, 0 : 2 * RB])
        nc.sync.dma_start(out=outp[:, :], in_=out_sb[:, :])
    return nc


def _morton(p, bits=10):
    q = np.minimum((p * (1 << bits)).astype(np.uint64), (1 << bits) - 1)
    code = np.zeros(len(p), np.uint64)
    for b in range(bits):
        for dim in range(3):
            code |= ((q[:, dim] >> np.uint64(b)) & np.uint64(1)) << np.uint64(3 * b + dim)
    return code


def _fp22(x):
    return (x.view(np.uint32) & np.uint32(0xFFFFFC00)).view(np.float32)


def _prep_batch(b, points, pointfea1, pointfea2, weights):
    perm = np.argsort(_morton(points[b]))
    inv = np.float32(1.0 / (SIGMA * SIGMA))
    p = points[b][perm]
    f1 = pointfea1[b][perm]
    f2 = pointfea2[b][perm]
    w = weights[b, :, 0][perm]

    p2 = (p * p).sum(1)
    f1sq = (f1 * f1).sum(1)
    f2sq = (f2 * f2).sum(1)
    onesN = np.ones((N, 1), np.float32)

    a_pts = np.concatenate([2.0 * p * inv, onesN, (p2 * inv)[:, None]], 1).astype(np.float32)
    b_pts = np.concatenate([p, -(p2 * inv)[:, None], -onesN], 1).astype(np.float32)
    a_fea = _fp22(np.concatenate([2.0 * f1, onesN, f1sq[:, None]], 1).astype(np.float32))
    b_fea = _fp22(np.concatenate([f2, -f2sq[:, None], -onesN], 1).astype(np.float32))
    a_fea_bf = a_fea.astype(ml_dtypes.bfloat16)
    b_fea_bf = b_fea.astype(ml_dtypes.bfloat16)
    return p, f1, f2, w, a_pts, b_pts, a_fea, b_fea, a_fea_bf, b_fea_bf


def make_in_maps(points, pointfea1, pointfea2, weights):
    points = np.asarray(points, np.float32)
    pointfea1 = np.asarray(pointfea1, np.float32)
    pointfea2 = np.asarray(pointfea2, np.float32)
    weights = np.asarray(weights, np.float32)

    batch_data = [
        _prep_batch(b, points, pointfea1, pointfea2, weights) for b in range(B)
    ]
    in_maps = []
    for k in range(NCORES):
        b = k // CPB
        r0 = (k % CPB) * ROWS
        p, f1, f2, w, a_pts, b_pts, a_fea, b_fea, a_fea_bf, b_fea_bf = batch_data[b]
        # per-row-block band starts (global j), gathered host-side
        bpt_band = np.empty((KP, RB * W), np.float32)
        bfe_band = np.empty((KF, RB * W), np.float32)
        for rb in range(RB):
            g0 = r0 + rb * 128
            s = min(max(g0 - PAD, 0), N - W)
            bpt_band[:, rb * W : (rb + 1) * W] = b_pts[s : s + W].T
            bfe_band[:, rb * W : (rb + 1) * W] = b_fea[s : s + W].T
        in_maps.append(
            {
                "aptT": np.ascontiguousarray(a_pts[r0 : r0 + ROWS].T),
                "bptb": bpt_band,
                "afeT": np.ascontiguousarray(a_fea_bf[r0 : r0 + ROWS].T),
                "bfeT": np.ascontiguousarray(b_fea_bf.T),
                "afeR": np.ascontiguousarray(a_fea[r0 : r0 + ROWS].T),
                "bfeb": bfe_band,
                "wcol": np.ascontiguousarray(w[r0 : r0 + ROWS].reshape(RB, 128).T),
                "f1r": np.ascontiguousarray(
                    f1[r0 : r0 + ROWS].reshape(RB, 128, D).transpose(1, 0, 2).reshape(128, RB * D)
                ),
                "f2r": np.ascontiguousarray(
                    f2[r0 : r0 + ROWS].reshape(RB, 128, D).transpose(1, 0, 2).reshape(128, RB * D)
                ),
            }
        )
    return in_maps


def get_nc():
    if "nc" not in _CACHE:
        nc = _build()
        nc.finalize()
        _CACHE["nc"] = nc
    return _CACHE["nc"]


def combine_partials(parts):
    """parts: [NCORES, 16] array of per-core (8 ce cols, 8 reg cols)."""
    parts = np.asarray(parts, np.float64)
    ce = parts[:, 0:RB].sum(1).reshape(B, CPB).sum(1)
    reg = parts[:, RB : 2 * RB].sum(1).reshape(B, CPB).sum(1) / (29.0 * N)
    return ce.astype(np.float32), reg.astype(np.float32)


def kernel(points, pointfea1, pointfea2, weights):
    nc = get_nc()
    in_maps = make_in_maps(points, pointfea1, pointfea2, weights)
    res = run_bass_kernel_spmd(nc, in_maps, core_ids=list(range(NCORES)))
    parts = np.stack([res.results[k]["partials"][0] for k in range(NCORES)])
    return combine_partials(parts)
